# revision 1
# baseline (speedup 1.0000x reference)
"""EncDec ConvLSTM kernel for 8 Trainium2 NeuronCores.

Sharding: 8 cores = 4 (batch) x 2 (spatial row-halves). Each core computes
its 32 output rows plus a shrinking redundant halo (21-s extra rows at
recurrent step s), so no cross-core communication is needed. Row-half 1
cores receive a vertically flipped image and ky-flipped conv weights, so a
single SPMD program serves all cores.

Conv3x3 is mapped to PE matmuls over pixels (N=512 free dim, fp32r):
per 8-row tile the 4H=256 gate channels come from 2 M-tiles x 7
accumulating matmuls (1 x-im2col K=72 + 3 paired h-taps K=128 + 3 single
h-taps K=64). The kx=0/kx=2 h-taps are packed into one K=128 matmul using
a column-shifted copy of h kept in partitions 64..127.
"""

import os
import sys

import numpy as np

for _p in ("/opt/trn_rl_repo", "/root/.axon_site/_ro/trn_rl_repo"):
    if os.path.isdir(_p) and _p not in sys.path:
        sys.path.append(_p)

T = 10
F = 8
HD = 64
HS = 64
WS = 64
NCORES = 8
PW = 66  # padded grid width/height
NSTEPS = 2 * T

_CACHE = {}


def _regions():
    """Rounded compute-region row counts per recurrent step s=1..NSTEPS."""
    out = []
    for s in range(1, NSTEPS + 1):
        need = NSTEPS + 1 - s
        rows = min(HS, 32 + need)
        rows = min(HS, ((rows + 7) // 8) * 8)
        out.append(rows)
    return out


def _build_program(use_bf16=True):
    from concourse import bacc, mybir, tile

    F32 = mybir.dt.float32
    MMDT = mybir.dt.bfloat16 if use_bf16 else mybir.dt.float32r
    ACT = mybir.ActivationFunctionType

    nc = bacc.Bacc("TRN2", target_bir_lowering=False, debug=False,
                   num_devices=NCORES)

    def din(name, shape, dt=MMDT):
        return nc.dram_tensor(name, shape, dt, kind="ExternalInput").ap()

    xe_d = din("xe", [T, F, PW, PW])
    xd_d = din("xd", [T, F, PW, PW])
    w_x = {"e": din("w_ex", [72, 256]), "d": din("w_dx", [72, 256])}
    w_p = {ph: [din(f"w_{ph}p{k}", [128, 256]) for k in range(3)]
           for ph in ("e", "d")}
    # middle-column (kx=1) taps packed as two K=96 groups:
    #   A = [tap(0,1) all 64 ch; tap(1,1) ch 0:32]   vs ta = [h; h row-shift]
    #   B = [tap(1,1) ch 32:64; tap(2,1) all 64 ch]  vs tb
    w_a = {ph: din(f"w_{ph}a", [96, 256]) for ph in ("e", "d")}
    w_b = {ph: din(f"w_{ph}b", [96, 256]) for ph in ("e", "d")}
    w_op = [din(f"w_op{k}", [128, 8]) for k in range(3)]
    w_oa = din("w_oa", [96, 8])
    w_ob = din("w_ob", [96, 8])
    b_m0 = {"e": din("b_e0", [128, 1], F32), "d": din("b_d0", [128, 1], F32)}
    b_m1 = {"e": din("b_e1", [128, 1], F32), "d": din("b_d1", [128, 1], F32)}
    b_o = din("b_o", [8, 1], F32)
    zz_d = din("zz", [128, PW * PW])  # fp32r zeros for state init
    y_d = nc.dram_tensor("y", [T, F, 32, WS], F32, kind="ExternalOutput").ap()

    regions = _regions()

    with tile.TileContext(nc) as tc:
        with tc.tile_pool(name="wpool", bufs=1) as wp, \
             tc.tile_pool(name="state", bufs=1) as stp, \
             tc.tile_pool(name="x2p", bufs=2) as x2p, \
             tc.tile_pool(name="gps", bufs=6, space="PSUM") as gps, \
             tc.tile_pool(name="ops", bufs=2, space="PSUM") as ops, \
             tc.tile_pool(name="fip", bufs=3) as fip, \
             tc.tile_pool(name="ogp", bufs=3) as ogp, \
             tc.tile_pool(name="t1p", bufs=3) as t1p, \
             tc.tile_pool(name="t1lp", bufs=3) as t1lp, \
             tc.tile_pool(name="thp", bufs=3) as thp, \
             tc.tile_pool(name="yyp", bufs=2) as yyp:

            # ---- load weights / biases into SBUF ----
            def wtile(src, shape, tag, dt=MMDT):
                t_ = wp.tile(shape, dt, tag=tag)
                nc.sync.dma_start(t_[:], src[:])
                return t_

            sw_x = {ph: wtile(w_x[ph], [72, 256], f"wx{ph}")
                    for ph in ("e", "d")}
            sw_p = {ph: [wtile(w_p[ph][k], [128, 256], f"wp{ph}{k}")
                         for k in range(3)] for ph in ("e", "d")}
            sw_a = {ph: wtile(w_a[ph], [96, 256], f"wa{ph}")
                    for ph in ("e", "d")}
            sw_b = {ph: wtile(w_b[ph], [96, 256], f"wb{ph}")
                    for ph in ("e", "d")}
            sw_op = [wtile(w_op[k], [128, 8], f"wop{k}") for k in range(3)]
            sw_oa = wtile(w_oa, [96, 8], "woa")
            sw_ob = wtile(w_ob, [96, 8], "wob")
            sb_m0 = {ph: wtile(b_m0[ph], [128, 1], f"b0{ph}", F32)
                     for ph in ("e", "d")}
            sb_m1 = {ph: wtile(b_m1[ph], [128, 1], f"b1{ph}", F32)
                     for ph in ("e", "d")}
            sb_o = wtile(b_o, [8, 1], "bo", F32)

            # ---- persistent state ----
            hhA = stp.tile([128, PW * PW], MMDT, tag="hhA")
            hhB = stp.tile([128, PW * PW], MMDT, tag="hhB")
            taA = stp.tile([96, PW * PW], MMDT, tag="taA")
            taB = stp.tile([96, PW * PW], MMDT, tag="taB")
            tbA = stp.tile([96, PW * PW], MMDT, tag="tbA")
            tbB = stp.tile([96, PW * PW], MMDT, tag="tbB")
            c_t = stp.tile([64, PW * PW], F32, tag="c")
            nc.sync.dma_start(hhA[:], zz_d[:])
            nc.sync.dma_start(hhB[:], zz_d[:])
            for t_ in (taA, taB, tbA, tbB):
                nc.sync.dma_start(t_[:], zz_d[0:96])
            nc.vector.memset(c_t[:], 0.0)

            # PE clock warm-up: a dense run of small-weight matmuls keeps
            # the PE array near-100% active so HAM raises the clock to
            # 2.4GHz before the real work starts. Gate matmuls alone never
            # warm it (128-col LDWEIGHTS between every MM lowers the
            # array's duty cycle below HAM's busy threshold).
            for _ in range(64):
                wu = ops.tile([8, 512], F32, tag="pso")
                nc.tensor.matmul(wu[:], sw_op[0][:], hhA[:, 0:512],
                                 start=True, stop=True)
            def gv(t_):
                return t_[:].rearrange("p (r c) -> p r c", c=PW)

            hhAv, hhBv = gv(hhA), gv(hhB)
            taAv, taBv, tbAv, tbBv = gv(taA), gv(taB), gv(tbA), gv(tbB)
            c_v = gv(c_t)

            def emit_x2col(s):
                """Load x im2col for step s: partition (ky*3+kx)*8+ic holds
                the flat padded image shifted by ky*66+kx (contiguous)."""
                ph = "e" if s <= T else "d"
                t_idx = (s - 1) if ph == "e" else (s - 1 - T)
                x_src = xe_d if ph == "e" else xd_d
                rp = regions[s - 1]
                ln = (rp - 1) * PW + 64
                x2 = x2p.tile([72, 57 * PW], MMDT, tag="x2")
                flat = x_src[t_idx].rearrange("a r c -> a (r c)")
                for tap in range(9):
                    sh = (tap // 3) * PW + (tap % 3)
                    nc.gpsimd.dma_start(x2[tap * 8:(tap + 1) * 8, 0:ln],
                                        flat[:, sh:sh + ln])
                return x2

            def emit_outconv(s, h_view, ta_view, tb_view):
                """relu(out conv) for decoder step s, reading its h buffer."""
                t_o = s - 1 - T
                for n2 in range(4):
                    r0 = n2 * 8
                    pso = ops.tile([8, 512], F32, tag="pso")
                    for k in range(3):
                        nc.tensor.matmul(pso[:], sw_op[k][:],
                                         h_view[:, r0 + k:r0 + k + 8, 0:64],
                                         start=(k == 0), stop=False)
                    nc.tensor.matmul(pso[:], sw_oa[:],
                                     ta_view[0:96, r0:r0 + 8, 1:65],
                                     start=False, stop=False)
                    nc.tensor.matmul(pso[:], sw_ob[:],
                                     tb_view[0:96, r0 + 1:r0 + 9, 1:65],
                                     start=False, stop=True)
                    yy = yyp.tile([8, 512], F32, tag="yy")
                    nc.scalar.activation(yy[:], pso[:], ACT.Relu,
                                         bias=sb_o[:])
                    nc.gpsimd.dma_start(
                        y_d[t_o, :, r0:r0 + 8, :],
                        yy[:].rearrange("p (r c) -> p r c", c=64))

            x2_cur = emit_x2col(1)
            for s in range(1, NSTEPS + 1):
                ph = "e" if s <= T else "d"
                rp = regions[s - 1]
                ntiles = rp // 8
                if s % 2 == 0:  # read buffers written at s-1
                    h_r, ta_r, tb_r = hhAv, taAv, tbAv
                    h_w, ta_w, tb_w = hhBv, taBv, tbBv
                else:
                    h_r, ta_r, tb_r = hhBv, taBv, tbBv
                    h_w, ta_w, tb_w = hhAv, taAv, tbAv

                if s > T + 1:
                    # prev decoder step's out conv; deps long resolved
                    emit_outconv(s - 1, h_r, ta_r, tb_r)
                x2v = x2_cur[:].rearrange("p (r c) -> p r c", c=PW)
                if s < NSTEPS:
                    x2_next = emit_x2col(s + 1)  # prefetch on gpsimd queue

                if 1 < s <= T:
                    # re-warm burst: dense small-weight matmuls (8-col
                    # LDWEIGHTS) keep/restore the PE clock; the gate matmul
                    # pattern alone (128-col LDWEIGHTS per MM) falls under
                    # HAM's busy threshold. Decoder steps get this for free
                    # from the out-conv blocks. Reads x2 (read-only here).
                    for _ in range(10):
                        wu = ops.tile([8, 512], F32, tag="pso")
                        nc.tensor.matmul(wu[:], sw_oa[0:64, :],
                                         x2v[0:64, 0:8, 0:64],
                                         start=True, stop=True)

                for n in range(ntiles):
                    r0 = n * 8
                    ps0 = gps.tile([128, 512], F32, tag="ps")
                    ps1 = gps.tile([128, 512], F32, tag="ps")
                    for m, ps in ((0, ps0), (1, ps1)):
                        ms = slice(m * 128, (m + 1) * 128)
                        nc.tensor.matmul(ps[:], sw_x[ph][:, ms],
                                         x2v[0:72, r0:r0 + 8, 0:64],
                                         start=True, stop=False)
                        for k in range(3):
                            nc.tensor.matmul(
                                ps[:], sw_p[ph][k][:, ms],
                                h_r[:, r0 + k:r0 + k + 8, 0:64],
                                start=False, stop=False)
                        nc.tensor.matmul(ps[:], sw_a[ph][:, ms],
                                         ta_r[0:96, r0:r0 + 8, 1:65],
                                         start=False, stop=False)
                        nc.tensor.matmul(ps[:], sw_b[ph][:, ms],
                                         tb_r[0:96, r0 + 1:r0 + 9, 1:65],
                                         start=False, stop=True)

                    # epilogue: M0=[f;i] M1=[o;g]
                    fi = fip.tile([128, 512], F32, tag="fi")
                    og = ogp.tile([128, 512], F32, tag="og")
                    nc.scalar.activation(fi[:], ps0[:], ACT.Sigmoid,
                                         bias=sb_m0[ph][:])
                    nc.scalar.activation(og[0:64], ps1[0:64], ACT.Sigmoid,
                                         bias=sb_m1[ph][0:64])
                    nc.scalar.activation(og[64:128], ps1[64:128], ACT.Tanh,
                                         bias=sb_m1[ph][64:128])
                    # t1 = sigmoid(i) * tanh(g) on partitions 64..127
                    t1 = t1p.tile([128, 512], F32, tag="t1")
                    nc.vector.tensor_mul(t1[64:128], fi[64:128], og[64:128])
                    # cross-partition move 64..127 -> 0..63
                    t1l = t1lp.tile([64, 512], F32, tag="t1l")
                    nc.sync.dma_start(t1l[:], t1[64:128])
                    t1lv = t1l[:].rearrange("p (r c) -> p r c", c=64)
                    cs = c_v[0:64, r0 + 1:r0 + 9, 1:65]
                    nc.vector.tensor_mul(cs, cs, fi[0:64].rearrange(
                        "p (r c) -> p r c", c=64))
                    nc.vector.tensor_add(cs, cs, t1lv)
                    th = thp.tile([64, 512], F32, tag="th")
                    thv = th[:].rearrange("p (r c) -> p r c", c=64)
                    nc.scalar.activation(thv, cs, ACT.Tanh)
                    # h = tanh(c) * sigmoid(o) -> base half of write buffer
                    nc.vector.tensor_mul(
                        h_w[0:64, r0 + 1:r0 + 9, 1:65], thv,
                        og[0:64].rearrange("p (r c) -> p r c", c=64))
                    # shifted copy (cols +2) into partitions 64..127
                    nc.sync.dma_start(
                        h_w[64:128, r0 + 1:r0 + 9, 0:64],
                        h_w[0:64, r0 + 1:r0 + 9, 2:66])
                    # fan out h into the K=96 middle-column group tiles:
                    # ta = [h ch0:64 (row 0-base); h ch0:32 shifted one row]
                    # tb = [h ch32:64 (row 1-base); h ch0:64 shifted one row]
                    nc.sync.dma_start(ta_w[0:64, r0 + 1:r0 + 9, 1:65],
                                      h_w[0:64, r0 + 1:r0 + 9, 1:65])
                    nc.sync.dma_start(ta_w[64:96, r0:r0 + 8, 1:65],
                                      h_w[0:32, r0 + 1:r0 + 9, 1:65])
                    nc.sync.dma_start(tb_w[0:32, r0 + 1:r0 + 9, 1:65],
                                      h_w[32:64, r0 + 1:r0 + 9, 1:65])
                    nc.sync.dma_start(tb_w[32:96, r0:r0 + 8, 1:65],
                                      h_w[0:64, r0 + 1:r0 + 9, 1:65])

                if s < NSTEPS:
                    x2_cur = x2_next

            # out conv for the final decoder step (NSTEPS is even -> B bufs)
            emit_outconv(NSTEPS, hhBv, taBv, tbBv)

    nc.compile()
    return nc


def _prep_core_inputs(core, enc_in, dec_in, enc_W, enc_b, dec_W, dec_b,
                      out_W, out_b, use_bf16=True):
    import ml_dtypes
    mm_np = ml_dtypes.bfloat16 if use_bf16 else np.float32
    b, half = core // 2, core % 2
    # gate permutation: [f, i, o, g]
    perm = np.concatenate([np.arange(0, 128), np.arange(192, 256),
                           np.arange(128, 192)])

    def prep_x(x):
        x = x[b]  # [T, F, 64, 64]
        if half:
            x = x[:, :, ::-1, :]
        xp = np.zeros((T, F, PW, PW), np.float32)
        xp[:, :, 1:65, 1:65] = x
        return np.ascontiguousarray(xp)

    def prep_gateW(W, bias):
        Wf = W[:, :, ::-1, :] if half else W
        Wp = np.ascontiguousarray(Wf[perm])  # [256, 72, 3, 3]
        bp = bias[perm].astype(np.float32)
        # x part: rows (ky*3+kx)*8+ic
        lx = Wp[:, :F].transpose(2, 3, 1, 0).reshape(72, 256)
        lp = [np.concatenate([Wp[:, F:, k, 0].T, Wp[:, F:, k, 2].T], axis=0)
              for k in range(3)]  # [128, 256]
        la = np.concatenate([Wp[:, F:, 0, 1].T, Wp[:, F:F + 32, 1, 1].T],
                            axis=0)  # [96, 256]
        lb = np.concatenate([Wp[:, F + 32:, 1, 1].T, Wp[:, F:, 2, 1].T],
                            axis=0)  # [96, 256]
        return (np.ascontiguousarray(lx),
                [np.ascontiguousarray(a) for a in lp],
                np.ascontiguousarray(la), np.ascontiguousarray(lb),
                np.ascontiguousarray(bp[0:128].reshape(128, 1)),
                np.ascontiguousarray(bp[128:256].reshape(128, 1)))

    ex, ep, ea, eb, eb0, eb1 = prep_gateW(enc_W, enc_b)
    dx, dp, da, db, db0, db1 = prep_gateW(dec_W, dec_b)
    oWf = out_W[:, :, ::-1, :] if half else out_W
    op = [np.ascontiguousarray(np.concatenate(
        [oWf[:, :, k, 0].T, oWf[:, :, k, 2].T], axis=0).astype(np.float32))
        for k in range(3)]
    oa = np.ascontiguousarray(np.concatenate(
        [oWf[:, :, 0, 1].T, oWf[:, 0:32, 1, 1].T], axis=0))
    ob = np.ascontiguousarray(np.concatenate(
        [oWf[:, 32:64, 1, 1].T, oWf[:, :, 2, 1].T], axis=0))

    m = {"xe": prep_x(enc_in), "xd": prep_x(dec_in),
         "w_ex": ex, "w_dx": dx,
         "w_ea": ea, "w_eb": eb, "w_da": da, "w_db": db,
         "w_oa": oa, "w_ob": ob,
         "b_e0": eb0, "b_e1": eb1, "b_d0": db0, "b_d1": db1,
         "b_o": np.ascontiguousarray(out_b.reshape(8, 1).astype(np.float32)),
         "zz": np.zeros((128, PW * PW), np.float32)}
    for k in range(3):
        m[f"w_ep{k}"] = ep[k]
        m[f"w_dp{k}"] = dp[k]
        m[f"w_op{k}"] = op[k]
    f32_keys = {"b_e0", "b_e1", "b_d0", "b_d1", "b_o"}
    return {k: np.ascontiguousarray(np.asarray(
        v, np.float32 if k in f32_keys else mm_np)) for k, v in m.items()}


def _install_trace_hook():
    """Shim antenv.axon_hooks for NTFF profiling (dev only)."""
    import contextlib
    import ctypes
    import types

    so = "/opt/axon/libaxon_pjrt.so"
    if "antenv.axon_hooks" in sys.modules or not os.path.exists(so):
        return
    lib = ctypes.CDLL(so)
    if not hasattr(lib, "axon_start_nrt_profile"):
        return
    lib.axon_start_nrt_profile.argtypes = [ctypes.POINTER(ctypes.c_int64),
                                           ctypes.c_size_t]
    lib.axon_start_nrt_profile.restype = ctypes.c_int64
    lib.axon_stop_nrt_profile.argtypes = [ctypes.c_char_p]
    lib.axon_stop_nrt_profile.restype = ctypes.c_int64

    def _mk():
        @contextlib.contextmanager
        def _hook(output_dir, device_ids):
            import jax
            jax.devices()
            if device_ids:
                ids = (ctypes.c_int64 * len(device_ids))(*device_ids)
                rc = lib.axon_start_nrt_profile(ids, len(device_ids))
            else:
                rc = lib.axon_start_nrt_profile(None, 0)
            if rc != 0:
                raise RuntimeError(f"axon_start_nrt_profile rc={rc}")
            try:
                yield
            finally:
                lib.axon_stop_nrt_profile(str(output_dir).encode())
        return _hook

    mod = types.ModuleType("antenv.axon_hooks")
    mod.get_axon_ntff_profile_hook = _mk
    sys.modules["antenv.axon_hooks"] = mod


def kernel(enc_in, dec_in, enc_W, enc_b, dec_W, dec_b, out_W, out_b):
    from concourse.bass_utils import run_bass_kernel_spmd

    trace = os.environ.get("KERNEL_TRACE", "") == "1"
    if trace:
        _install_trace_hook()

    use_bf16 = os.environ.get("KERNEL_DTYPE", "bf16") != "f32r"
    if "nc" not in _CACHE:
        _CACHE["nc"] = _build_program(use_bf16)
    nc = _CACHE["nc"]

    args = (np.asarray(enc_in, np.float32), np.asarray(dec_in, np.float32),
            np.asarray(enc_W, np.float32), np.asarray(enc_b, np.float32),
            np.asarray(dec_W, np.float32), np.asarray(dec_b, np.float32),
            np.asarray(out_W, np.float32), np.asarray(out_b, np.float32))
    in_maps = [_prep_core_inputs(c, *args, use_bf16=use_bf16)
               for c in range(NCORES)]

    res = run_bass_kernel_spmd(nc, in_maps, list(range(NCORES)), trace=trace)
    if trace:
        _CACHE["exec_time_ns"] = res.exec_time_ns

    B = enc_in.shape[0]
    out = np.empty((B, T, F, HS, WS), np.float32)
    for c in range(NCORES):
        b, half = c // 2, c % 2
        yc = res.results[c]["y"]  # [T, F, 32, 64]
        if half:
            out[b, :, :, 32:64, :] = yc[:, :, ::-1, :]
        else:
            out[b, :, :, 0:32, :] = yc
    return out



# revision 4
# speedup vs baseline: 1.3530x; 1.3530x over previous
"""EncDec ConvLSTM kernel for 8 Trainium2 NeuronCores.

Sharding: 8 cores = 4 (batch) x 2 (spatial row-halves). Each core computes
its 32 output rows plus a shrinking redundant halo (exact: 53-s rows at
recurrent step s), so no cross-core communication is needed. Row-half 1
cores receive a vertically flipped image and ky-flipped conv weights, so a
single SPMD program serves all cores.

Conv3x3 maps to PE matmuls over pixels (N = rows*64 free dim, bf16).
Per 8-row tile the 4H=256 gate channels come from 2 M-tiles x 6
accumulating matmuls: 1 x-im2col (K=72, im2col prebuilt on host, one DMA
per step), 3 kx-pair taps (K=128, via hh = [h; h shifted 2 cols]),
1 mid-column pair (K=128, via hh2 = [h; h shifted 2 rows]) and 1 single
mid tap (K=64, read from hh lower half directly). State fan-out is 3
contiguous SBUF->SBUF band copies per tile, split across the two HWDGE
queues (sync + scalar) so no single DMA queue serializes the recurrence.
The out conv runs 4 row-blocks concurrently via PE column tiling.
"""

import os
import sys

import numpy as np

for _p in ("/opt/trn_rl_repo", "/root/.axon_site/_ro/trn_rl_repo"):
    if os.path.isdir(_p) and _p not in sys.path:
        sys.path.append(_p)

T = 10
F = 8
HD = 64
HS = 64
WS = 64
NCORES = 8
PW = 66  # padded grid width/height
NSTEPS = 2 * T
X2LEN = 57 * PW  # prebuilt im2col free length

_CACHE = {}


def _regions():
    """Exact compute-region row counts per recurrent step s=1..NSTEPS."""
    return [min(HS, 53 - s) for s in range(1, NSTEPS + 1)]


def _build_program(use_bf16=True):
    from concourse import bacc, mybir, tile

    F32 = mybir.dt.float32
    MMDT = mybir.dt.bfloat16 if use_bf16 else mybir.dt.float32r
    ACT = mybir.ActivationFunctionType
    ALU = mybir.AluOpType

    nc = bacc.Bacc("TRN2", target_bir_lowering=False, debug=False,
                   num_devices=NCORES)

    def din(name, shape, dt=MMDT):
        return nc.dram_tensor(name, shape, dt, kind="ExternalInput").ap()

    x2_d = din("x2", [NSTEPS, 72, X2LEN])
    w_x = {ph: din(f"w_{ph}x", [72, 256]) for ph in ("e", "d")}
    w_p = {ph: [din(f"w_{ph}p{k}", [128, 256]) for k in range(3)]
           for ph in ("e", "d")}
    w_m = {ph: din(f"w_{ph}m", [128, 256]) for ph in ("e", "d")}
    w_s = {ph: din(f"w_{ph}s", [64, 256]) for ph in ("e", "d")}
    w_op = [din(f"w_op{k}", [128, 8]) for k in range(3)]
    w_om = din("w_om", [128, 8])
    w_os = din("w_os", [64, 8])
    b_m0 = {ph: din(f"b_{ph}0", [128, 1], F32) for ph in ("e", "d")}
    b_m1 = {ph: din(f"b_{ph}1", [128, 1], F32) for ph in ("e", "d")}
    b_o = din("b_o", [128, 1], F32)
    y_d = nc.dram_tensor("y", [T, F, 32, WS], F32, kind="ExternalOutput").ap()

    regions = _regions()

    with tile.TileContext(nc) as tc:
        with tc.tile_pool(name="wpool", bufs=1) as wp, \
             tc.tile_pool(name="state", bufs=1) as stp, \
             tc.tile_pool(name="x2p", bufs=2) as x2p, \
             tc.tile_pool(name="gps", bufs=4, space="PSUM") as gps, \
             tc.tile_pool(name="ops", bufs=4, space="PSUM") as ops, \
             tc.tile_pool(name="fip", bufs=3) as fip, \
             tc.tile_pool(name="ogp", bufs=3) as ogp, \
             tc.tile_pool(name="t1p", bufs=3) as t1p, \
             tc.tile_pool(name="t1lp", bufs=3) as t1lp, \
             tc.tile_pool(name="thp", bufs=3) as thp, \
             tc.tile_pool(name="yyp", bufs=2) as yyp:

            # ---- weights / biases into SBUF, split across the 2 HW queues
            def wtile(src, shape, tag, dt=MMDT, q=None):
                t_ = wp.tile(shape, dt, tag=tag)
                (q or nc.sync).dma_start(t_[:], src[:])
                return t_

            sw_x = {ph: wtile(w_x[ph], [72, 256], f"wx{ph}")
                    for ph in ("e", "d")}
            sw_p = {"e": [wtile(w_p["e"][k], [128, 256], f"wpe{k}")
                          for k in range(3)],
                    "d": [wtile(w_p["d"][k], [128, 256], f"wpd{k}",
                                q=nc.scalar) for k in range(3)]}
            sw_m = {ph: wtile(w_m[ph], [128, 256], f"wm{ph}", q=nc.scalar)
                    for ph in ("e", "d")}
            sw_s = {ph: wtile(w_s[ph], [64, 256], f"ws{ph}", q=nc.scalar)
                    for ph in ("e", "d")}
            sw_op = [wtile(w_op[k], [128, 8], f"wop{k}", q=nc.scalar)
                     for k in range(3)]
            sw_om = wtile(w_om, [128, 8], "wom", q=nc.scalar)
            sw_os = wtile(w_os, [64, 8], "wos", q=nc.scalar)
            sb_m0 = {ph: wtile(b_m0[ph], [128, 1], f"b0{ph}", F32)
                     for ph in ("e", "d")}
            sb_m1 = {ph: wtile(b_m1[ph], [128, 1], f"b1{ph}", F32)
                     for ph in ("e", "d")}
            sb_o = wtile(b_o, [128, 1], "bo", F32, q=nc.scalar)

            # ---- persistent state ----
            hhA = stp.tile([128, PW * PW], MMDT, tag="hhA")
            hhB = stp.tile([128, PW * PW], MMDT, tag="hhB")
            h2A = stp.tile([128, PW * PW], MMDT, tag="h2A")
            h2B = stp.tile([128, PW * PW], MMDT, tag="h2B")
            c_t = stp.tile([64, HS * 64], F32, tag="c")

            def gv(t_):
                return t_[:].rearrange("p (r c) -> p r c", c=PW)

            # Zero only the borders that are read but never written:
            # row 0 everywhere; cols 0 and 65 of the h (lower) halves.
            for t_ in (hhA, hhB, h2A, h2B):
                v = gv(t_)
                nc.vector.memset(v[0:128, 0:1, 0:PW], 0.0)
            for t_ in (hhA, hhB):
                v = gv(t_)
                nc.vector.memset(v[0:64, 0:PW, 0:1], 0.0)
                nc.vector.memset(v[0:64, 0:PW, 65:66], 0.0)

            def load_x2(s):
                rp = regions[s - 1]
                ln = (rp - 1) * PW + 64
                x2 = x2p.tile([72, X2LEN], MMDT, tag="x2")
                nc.scalar.dma_start(x2[:, 0:ln], x2_d[s - 1][:, 0:ln])
                return x2

            def emit_outconv(s, hv, h2v):
                """relu(out conv + bias) for decoder step s; 4 row-blocks
                run concurrently in distinct PE column groups."""
                t_o = s - 1 - T
                psos = [ops.tile([128, 512], F32, tag="pso",
                                 name=f"pso{j}") for j in range(4)]
                yy = yyp.tile([128, 512], F32, tag="yy")
                for k in range(3):
                    for j in range(4):
                        r0 = 8 * j
                        nc.tensor.matmul(psos[j][32 * j:32 * j + 8, :],
                                         sw_op[k][:],
                                         hv[:, r0 + k:r0 + k + 8, 0:64],
                                         start=(k == 0), stop=False,
                                         tile_position=(0, 32 * j))
                for j in range(4):
                    r0 = 8 * j
                    nc.tensor.matmul(psos[j][32 * j:32 * j + 8, :], sw_om[:],
                                     h2v[:, r0:r0 + 8, 1:65],
                                     start=False, stop=False,
                                     tile_position=(0, 32 * j))
                for j in range(4):
                    r0 = 8 * j
                    nc.tensor.matmul(psos[j][32 * j:32 * j + 8, :], sw_os[:],
                                     hv[0:64, r0 + 1:r0 + 9, 1:65],
                                     start=False, stop=True,
                                     tile_position=(0, 32 * j))
                for j in range(4):
                    sl = slice(32 * j, 32 * j + 8)
                    nc.vector.tensor_scalar(
                        out=yy[sl, :], in0=psos[j][sl, :],
                        scalar1=sb_o[sl], scalar2=0.0,
                        op0=ALU.add, op1=ALU.max)
                    nc.gpsimd.dma_start(
                        y_d[t_o, :, 8 * j:8 * j + 8, :],
                        yy[sl].rearrange("p (r c) -> p r c", c=64))

            def gate_tile(s, ph, r0, rows, hv_r, h2v_r, h_w, h2_w, hv_w,
                          x2v):
                N = rows * 64
                first = (s == 1)  # h == 0: x conv + bias only
                ps0 = gps.tile([128, N], F32, tag="ps")
                ps1 = gps.tile([128, N], F32, tag="ps")
                for m, ps in ((0, ps0), (1, ps1)):
                    ms = slice(m * 128, (m + 1) * 128)
                    nc.tensor.matmul(ps[:], sw_x[ph][:, ms],
                                     x2v[0:72, r0:r0 + rows, 0:64],
                                     start=True, stop=first)
                    if not first:
                        for k in range(3):
                            nc.tensor.matmul(
                                ps[:], sw_p[ph][k][:, ms],
                                hv_r[:, r0 + k:r0 + k + rows, 0:64],
                                start=False, stop=False)
                        nc.tensor.matmul(ps[:], sw_m[ph][:, ms],
                                         h2v_r[:, r0:r0 + rows, 1:65],
                                         start=False, stop=False)
                        nc.tensor.matmul(
                            ps[:], sw_s[ph][:, ms],
                            hv_r[0:64, r0 + 1:r0 + 1 + rows, 1:65],
                            start=False, stop=True)

                # epilogue: M0=[f;i] M1=[o;g]
                fi = fip.tile([128, N], F32, tag="fi")
                og = ogp.tile([128, N], F32, tag="og")
                nc.scalar.activation(fi[:], ps0[:], ACT.Sigmoid,
                                     bias=sb_m0[ph][:])
                nc.scalar.activation(og[0:64], ps1[0:64], ACT.Sigmoid,
                                     bias=sb_m1[ph][0:64])
                nc.scalar.activation(og[64:128], ps1[64:128], ACT.Tanh,
                                     bias=sb_m1[ph][64:128])
                # t1 = sigmoid(i) * tanh(g) on partitions 64..127
                t1 = t1p.tile([128, N], F32, tag="t1")
                nc.vector.tensor_mul(t1[64:128], fi[64:128], og[64:128])
                cs = c_t[:, r0 * 64:r0 * 64 + N]  # [64, N] contiguous
                if first:
                    # c = t1 (cross-partition move writes c directly)
                    nc.sync.dma_start(cs, t1[64:128])
                else:
                    t1l = t1lp.tile([64, N], F32, tag="t1l")
                    nc.sync.dma_start(t1l[:], t1[64:128])
                    nc.vector.tensor_mul(cs, cs, fi[0:64])
                    nc.vector.tensor_add(cs, cs, t1l[:])
                th = thp.tile([64, N], F32, tag="th")
                nc.scalar.activation(th[:], cs, ACT.Tanh)
                # h = tanh(c) * sigmoid(o) -> write buffer, padded layout
                nc.vector.tensor_mul(
                    hv_w[0:64, r0 + 1:r0 + 1 + rows, 1:65],
                    th[:].rearrange("p (r c) -> p r c", c=64),
                    og[0:64].rearrange("p (r c) -> p r c", c=64))
                # state fan-out: contiguous flat band copies
                b0 = (r0 + 1) * PW
                L = rows * PW
                hwf, h2f = h_w[:], h2_w[:]
                # hh upper: h shifted +2 cols (junk in pad cols, never read)
                nc.sync.dma_start(hwf[64:128, b0:b0 + L - 2],
                                  hwf[0:64, b0 + 2:b0 + L])
                # hh2 lower: plain copy of h
                nc.scalar.dma_start(h2f[0:64, b0:b0 + L],
                                    hwf[0:64, b0:b0 + L])
                # hh2 upper: h shifted +2 rows (band lands 2 rows up)
                if r0 == 0:
                    # dst band [b0-2PW, ...) clipped at 0: drop PW elems
                    nc.scalar.dma_start(h2f[64:128, 0:L - PW],
                                        hwf[0:64, b0 + PW:b0 + L])
                else:
                    nc.scalar.dma_start(
                        h2f[64:128, b0 - 2 * PW:b0 + L - 2 * PW],
                        hwf[0:64, b0:b0 + L])

            x2_cur = load_x2(1)
            for s in range(1, NSTEPS + 1):
                ph = "e" if s <= T else "d"
                rp = regions[s - 1]
                if s % 2 == 0:  # read buffers written at s-1
                    h_r, h2_r, h_w, h2_w = hhA, h2A, hhB, h2B
                else:
                    h_r, h2_r, h_w, h2_w = hhB, h2B, hhA, h2A
                hv_r, h2v_r, hv_w = gv(h_r), gv(h2_r), gv(h_w)

                if s > T + 1:
                    # prev decoder step's out conv; deps long resolved
                    emit_outconv(s - 1, hv_r, h2v_r)
                x2v = x2_cur[:].rearrange("p (r c) -> p r c", c=PW)
                if s < NSTEPS:
                    x2_next = load_x2(s + 1)  # prefetch on scalar queue

                r0 = 0
                while r0 < rp:
                    rows = min(8, rp - r0)
                    gate_tile(s, ph, r0, rows, hv_r, h2v_r, h_w, h2_w,
                              hv_w, x2v)
                    r0 += 8

                if s < NSTEPS:
                    x2_cur = x2_next

            # out conv for the final decoder step (NSTEPS even -> B bufs)
            emit_outconv(NSTEPS, gv(hhB), gv(h2B))

    nc.compile()
    return nc


def _prep_core_inputs(core, enc_in, dec_in, enc_W, enc_b, dec_W, dec_b,
                      out_W, out_b, use_bf16=True):
    import ml_dtypes
    mm_np = ml_dtypes.bfloat16 if use_bf16 else np.float32
    b, half = core // 2, core % 2
    # gate permutation: [f, i, o, g]
    perm = np.concatenate([np.arange(0, 128), np.arange(192, 256),
                           np.arange(128, 192)])

    def prep_x2(x):
        x = x[b]  # [T, F, 64, 64]
        if half:
            x = x[:, :, ::-1, :]
        xp = np.zeros((T, F, PW, PW), np.float32)
        xp[:, :, 1:65, 1:65] = x
        flat = xp.reshape(T, F, PW * PW)
        x2 = np.empty((T, 72, X2LEN), np.float32)
        for tap in range(9):
            sh = (tap // 3) * PW + (tap % 3)
            x2[:, tap * 8:(tap + 1) * 8, :] = flat[:, :, sh:sh + X2LEN]
        return x2

    def prep_gateW(W, bias):
        Wf = W[:, :, ::-1, :] if half else W
        Wp = np.ascontiguousarray(Wf[perm])  # [256, 72, 3, 3]
        bp = bias[perm].astype(np.float32)
        lx = Wp[:, :F].transpose(2, 3, 1, 0).reshape(72, 256)
        lp = [np.concatenate([Wp[:, F:, k, 0].T, Wp[:, F:, k, 2].T], axis=0)
              for k in range(3)]  # [128, 256]
        lm = np.concatenate([Wp[:, F:, 0, 1].T, Wp[:, F:, 2, 1].T],
                            axis=0)  # [128, 256]
        ls = Wp[:, F:, 1, 1].T  # [64, 256]
        return (np.ascontiguousarray(lx),
                [np.ascontiguousarray(a) for a in lp],
                np.ascontiguousarray(lm), np.ascontiguousarray(ls),
                np.ascontiguousarray(bp[0:128].reshape(128, 1)),
                np.ascontiguousarray(bp[128:256].reshape(128, 1)))

    ex, ep, em, es, eb0, eb1 = prep_gateW(enc_W, enc_b)
    dx, dp, dm, ds, db0, db1 = prep_gateW(dec_W, dec_b)
    oWf = out_W[:, :, ::-1, :] if half else out_W
    op = [np.ascontiguousarray(np.concatenate(
        [oWf[:, :, k, 0].T, oWf[:, :, k, 2].T], axis=0))
        for k in range(3)]  # [128, 8]
    om = np.ascontiguousarray(np.concatenate(
        [oWf[:, :, 0, 1].T, oWf[:, :, 2, 1].T], axis=0))  # [128, 8]
    osg = np.ascontiguousarray(oWf[:, :, 1, 1].T)  # [64, 8]
    bo = np.zeros((128, 1), np.float32)
    for j in range(4):
        bo[32 * j:32 * j + 8, 0] = out_b

    x2_all = np.concatenate([prep_x2(enc_in), prep_x2(dec_in)], axis=0)

    m = {"x2": x2_all,
         "w_ex": ex, "w_dx": dx,
         "w_em": em, "w_dm": dm, "w_es": es, "w_ds": ds,
         "w_om": om, "w_os": osg,
         "b_e0": eb0, "b_e1": eb1, "b_d0": db0, "b_d1": db1,
         "b_o": bo}
    for k in range(3):
        m[f"w_ep{k}"] = ep[k]
        m[f"w_dp{k}"] = dp[k]
        m[f"w_op{k}"] = op[k]
    f32_keys = {"b_e0", "b_e1", "b_d0", "b_d1", "b_o"}
    return {k: np.ascontiguousarray(np.asarray(
        v, np.float32 if k in f32_keys else mm_np)) for k, v in m.items()}


def _install_trace_hook():
    """Shim antenv.axon_hooks for NTFF profiling (dev only)."""
    import contextlib
    import ctypes
    import types

    so = "/opt/axon/libaxon_pjrt.so"
    if "antenv.axon_hooks" in sys.modules or not os.path.exists(so):
        return
    lib = ctypes.CDLL(so)
    if not hasattr(lib, "axon_start_nrt_profile"):
        return
    lib.axon_start_nrt_profile.argtypes = [ctypes.POINTER(ctypes.c_int64),
                                           ctypes.c_size_t]
    lib.axon_start_nrt_profile.restype = ctypes.c_int64
    lib.axon_stop_nrt_profile.argtypes = [ctypes.c_char_p]
    lib.axon_stop_nrt_profile.restype = ctypes.c_int64

    def _mk():
        @contextlib.contextmanager
        def _hook(output_dir, device_ids):
            import jax
            jax.devices()
            if device_ids:
                ids = (ctypes.c_int64 * len(device_ids))(*device_ids)
                rc = lib.axon_start_nrt_profile(ids, len(device_ids))
            else:
                rc = lib.axon_start_nrt_profile(None, 0)
            if rc != 0:
                raise RuntimeError(f"axon_start_nrt_profile rc={rc}")
            try:
                yield
            finally:
                lib.axon_stop_nrt_profile(str(output_dir).encode())
        return _hook

    mod = types.ModuleType("antenv.axon_hooks")
    mod.get_axon_ntff_profile_hook = _mk
    sys.modules["antenv.axon_hooks"] = mod


def kernel(enc_in, dec_in, enc_W, enc_b, dec_W, dec_b, out_W, out_b):
    from concourse.bass_utils import run_bass_kernel_spmd

    trace = os.environ.get("KERNEL_TRACE", "") == "1"
    if trace:
        _install_trace_hook()

    use_bf16 = os.environ.get("KERNEL_DTYPE", "bf16") != "f32r"
    if "nc" not in _CACHE:
        _CACHE["nc"] = _build_program(use_bf16)
    nc = _CACHE["nc"]

    args = (np.asarray(enc_in, np.float32), np.asarray(dec_in, np.float32),
            np.asarray(enc_W, np.float32), np.asarray(enc_b, np.float32),
            np.asarray(dec_W, np.float32), np.asarray(dec_b, np.float32),
            np.asarray(out_W, np.float32), np.asarray(out_b, np.float32))
    in_maps = [_prep_core_inputs(c, *args, use_bf16=use_bf16)
               for c in range(NCORES)]

    res = run_bass_kernel_spmd(nc, in_maps, list(range(NCORES)), trace=trace)
    if trace:
        _CACHE["exec_time_ns"] = res.exec_time_ns

    B = enc_in.shape[0]
    out = np.empty((B, T, F, HS, WS), np.float32)
    for c in range(NCORES):
        b, half = c // 2, c % 2
        yc = res.results[c]["y"]  # [T, F, 32, 64]
        if half:
            out[b, :, :, 32:64, :] = yc[:, :, ::-1, :]
        else:
            out[b, :, :, 0:32, :] = yc
    return out


# revision 6
# speedup vs baseline: 1.4824x; 1.0957x over previous
"""EncDec ConvLSTM kernel for 8 Trainium2 NeuronCores.

Sharding: 8 cores = 4 (batch) x 2 (spatial row-halves). Each core computes
its 32 output rows plus a shrinking redundant halo (exact: 53-s rows at
recurrent step s), so no cross-core communication is needed. Row-half 1
cores receive a vertically flipped image and ky-flipped conv weights, so a
single SPMD program serves all cores.

Conv3x3 maps to PE matmuls over pixels (N = rows*64 free dim, bf16).
Per 8-row tile the 4H=256 gate channels come from 2 M-tiles x 6
accumulating matmuls: 1 x-im2col (K=72, im2col prebuilt on host, one DMA
per step), 3 kx-pair taps (K=128, via hh = [h; h shifted 2 cols]),
1 mid-column pair (K=128, via hh2 = [h; h shifted 2 rows]) and 1 single
mid tap (K=64, read from hh lower half directly).

The pointwise epilogue is software-pipelined with a 1-tile skew (phase1 =
gate activations + c update inputs, phase2 = tanh(c), h write, state
fan-out) so no engine queue head-of-line blocks. tanh(g) is computed as
2*sigmoid(2g)-1 with g-weights doubled on the host, merging the o/g
activations into one 128-partition sigmoid. DMA queues are dedicated:
sync = cross-partition t1 move, scalar = the 3 contiguous state band
copies, gpsimd = x-im2col + y output.
"""

import os
import sys

import numpy as np

for _p in ("/opt/trn_rl_repo", "/root/.axon_site/_ro/trn_rl_repo"):
    if os.path.isdir(_p) and _p not in sys.path:
        sys.path.append(_p)

T = 10
F = 8
HD = 64
HS = 64
WS = 64
NCORES = 8
PW = 66  # padded grid width/height
NSTEPS = 2 * T
X2LEN = 57 * PW  # prebuilt im2col free length

# big packed weight tensor: 12 x 256-col gate blocks + 5 x 8-col out blocks
_GBLK = ["xe", "xd", "pe0", "pe1", "pe2", "pd0", "pd1", "pd2",
         "me", "md", "se", "sd"]
_OBLK = ["op0", "op1", "op2", "om", "os"]
_WCOLS = 256 * len(_GBLK) + 8 * len(_OBLK)

_CACHE = {}


def _regions():
    """Exact compute-region row counts per recurrent step s=1..NSTEPS."""
    return [min(HS, 53 - s) for s in range(1, NSTEPS + 1)]


def _build_program(use_bf16=True):
    from concourse import bacc, mybir, tile

    F32 = mybir.dt.float32
    MMDT = mybir.dt.bfloat16 if use_bf16 else mybir.dt.float32r
    ACT = mybir.ActivationFunctionType
    ALU = mybir.AluOpType

    nc = bacc.Bacc("TRN2", target_bir_lowering=False, debug=False,
                   num_devices=NCORES)

    x2_d = nc.dram_tensor("x2", [NSTEPS, 72, X2LEN], MMDT,
                          kind="ExternalInput").ap()
    wall_d = nc.dram_tensor("wall", [128, _WCOLS], MMDT,
                            kind="ExternalInput").ap()
    ball_d = nc.dram_tensor("ball", [128, 5], F32,
                            kind="ExternalInput").ap()
    y_d = nc.dram_tensor("y", [T, F, 32, WS], F32, kind="ExternalOutput").ap()

    regions = _regions()

    with tile.TileContext(nc) as tc:
        with tc.tile_pool(name="wpool", bufs=1) as wp, \
             tc.tile_pool(name="state", bufs=1) as stp, \
             tc.tile_pool(name="x2p", bufs=3) as x2p, \
             tc.tile_pool(name="gps", bufs=4, space="PSUM") as gps, \
             tc.tile_pool(name="ops", bufs=4, space="PSUM") as ops, \
             tc.tile_pool(name="fip", bufs=3) as fip, \
             tc.tile_pool(name="ogp", bufs=4) as ogp, \
             tc.tile_pool(name="t1p", bufs=3) as t1p, \
             tc.tile_pool(name="t1lp", bufs=3) as t1lp, \
             tc.tile_pool(name="thp", bufs=3) as thp, \
             tc.tile_pool(name="yyp", bufs=2) as yyp:

            # ---- packed weights: two big DMAs, one per HWDGE queue ----
            wall = wp.tile([128, _WCOLS], MMDT, tag="wall")
            half = 256 * 6
            nc.sync.dma_start(wall[:, 0:half], wall_d[:, 0:half])
            nc.scalar.dma_start(wall[:, half:_WCOLS], wall_d[:, half:_WCOLS])
            ball = wp.tile([128, 5], F32, tag="ball")
            nc.sync.dma_start(ball[:], ball_d[:])

            goff = {k: 256 * i for i, k in enumerate(_GBLK)}
            ooff = {k: 256 * len(_GBLK) + 8 * i for i, k in enumerate(_OBLK)}

            def gw(key, m, kdim=128):
                o = goff[key] + 128 * m
                return wall[0:kdim, o:o + 128]

            def ow(key, kdim=128):
                o = ooff[key]
                return wall[0:kdim, o:o + 8]

            sb_b = {("e", 0): ball[:, 0:1], ("e", 1): ball[:, 1:2],
                    ("d", 0): ball[:, 2:3], ("d", 1): ball[:, 3:4]}
            sb_o = ball[:, 4:5]

            # ---- persistent state ----
            hhA = stp.tile([128, PW * PW], MMDT, tag="hhA")
            hhB = stp.tile([128, PW * PW], MMDT, tag="hhB")
            h2A = stp.tile([128, PW * PW], MMDT, tag="h2A")
            h2B = stp.tile([128, PW * PW], MMDT, tag="h2B")
            c_t = stp.tile([64, HS * 64], F32, tag="c")

            def gv(t_):
                return t_[:].rearrange("p (r c) -> p r c", c=PW)

            # Zero only the borders that are read but never written:
            # row 0 everywhere; cols 0 and 65 of the h (lower) halves.
            for t_ in (hhA, hhB, h2A, h2B):
                v = gv(t_)
                nc.vector.memset(v[0:128, 0:1, 0:PW], 0.0)
            for t_ in (hhA, hhB):
                v = gv(t_)
                nc.vector.memset(v[0:64, 0:PW, 0:1], 0.0)
                nc.vector.memset(v[0:64, 0:PW, 65:66], 0.0)

            def load_x2(s):
                rp = regions[s - 1]
                ln = (rp - 1) * PW + 64
                x2 = x2p.tile([72, X2LEN], MMDT, tag="x2")
                nc.gpsimd.dma_start(x2[:, 0:ln], x2_d[s - 1][:, 0:ln])
                return x2

            def emit_outconv(s, hv, h2v):
                """relu(out conv + bias) for decoder step s; 4 row-blocks
                run concurrently in distinct PE column groups."""
                t_o = s - 1 - T
                psos = [ops.tile([128, 512], F32, tag="pso",
                                 name=f"pso{j}") for j in range(4)]
                yy = yyp.tile([128, 512], F32, tag="yy")
                for k in range(3):
                    for j in range(4):
                        r0 = 8 * j
                        nc.tensor.matmul(psos[j][32 * j:32 * j + 8, :],
                                         ow(f"op{k}"),
                                         hv[:, r0 + k:r0 + k + 8, 0:64],
                                         start=(k == 0), stop=False,
                                         tile_position=(0, 32 * j))
                for j in range(4):
                    r0 = 8 * j
                    nc.tensor.matmul(psos[j][32 * j:32 * j + 8, :], ow("om"),
                                     h2v[:, r0:r0 + 8, 1:65],
                                     start=False, stop=False,
                                     tile_position=(0, 32 * j))
                for j in range(4):
                    r0 = 8 * j
                    nc.tensor.matmul(psos[j][32 * j:32 * j + 8, :],
                                     ow("os", 64),
                                     hv[0:64, r0 + 1:r0 + 9, 1:65],
                                     start=False, stop=True,
                                     tile_position=(0, 32 * j))
                for j in range(4):
                    sl = slice(32 * j, 32 * j + 8)
                    nc.vector.tensor_scalar(
                        out=yy[sl, :], in0=psos[j][sl, :],
                        scalar1=sb_o[sl], scalar2=0.0,
                        op0=ALU.add, op1=ALU.max)
                    nc.gpsimd.dma_start(
                        y_d[t_o, :, 8 * j:8 * j + 8, :],
                        yy[sl].rearrange("p (r c) -> p r c", c=64))

            def gate_phase1(s, ph, r0, rows, hv_r, h2v_r, x2v):
                """Gate matmuls + activations + c-update inputs."""
                N = rows * 64
                first = (s == 1)  # h == 0: x conv + bias only
                ps0 = gps.tile([128, N], F32, tag="ps")
                ps1 = gps.tile([128, N], F32, tag="ps")
                for m, ps in ((0, ps0), (1, ps1)):
                    nc.tensor.matmul(ps[:], gw("x" + ph, m, 72),
                                     x2v[0:72, r0:r0 + rows, 0:64],
                                     start=True, stop=first)
                    if not first:
                        for k in range(3):
                            nc.tensor.matmul(
                                ps[:], gw(f"p{ph}{k}", m),
                                hv_r[:, r0 + k:r0 + k + rows, 0:64],
                                start=False, stop=False)
                        nc.tensor.matmul(ps[:], gw("m" + ph, m),
                                         h2v_r[:, r0:r0 + rows, 1:65],
                                         start=False, stop=False)
                        nc.tensor.matmul(
                            ps[:], gw("s" + ph, m, 64),
                            hv_r[0:64, r0 + 1:r0 + 1 + rows, 1:65],
                            start=False, stop=True)

                # M0=[f;i] M1=[o;2g] (g pre-act doubled via host weights)
                fi = fip.tile([128, N], F32, tag="fi")
                og = ogp.tile([128, N], F32, tag="og")
                nc.scalar.activation(fi[:], ps0[:], ACT.Sigmoid,
                                     bias=sb_b[(ph, 0)])
                nc.scalar.activation(og[:], ps1[:], ACT.Sigmoid,
                                     bias=sb_b[(ph, 1)])
                cs = c_t[:, r0 * 64:r0 * 64 + N]  # [64, N] contiguous
                if not first:
                    nc.vector.tensor_mul(cs, cs, fi[0:64])  # c *= sig(f)
                # t1 = sig(i) * tanh(g);  tanh(g) = 2*sig(2g) - 1
                t1 = t1p.tile([128, N], F32, tag="t1")
                nc.vector.tensor_scalar(
                    out=t1[64:128], in0=og[64:128], scalar1=2.0, scalar2=1.0,
                    op0=ALU.mult, op1=ALU.subtract)
                nc.vector.tensor_mul(t1[64:128], t1[64:128], fi[64:128])
                if first:
                    # c = t1 (cross-partition move writes c directly)
                    nc.sync.dma_start(cs, t1[64:128])
                    t1l = None
                else:
                    t1l = t1lp.tile([64, N], F32, tag="t1l")
                    nc.sync.dma_start(t1l[:], t1[64:128])
                return (s, r0, rows, og, t1l)

            def gate_phase2(ctx, h_w, h2_w, hv_w):
                """c += t1, tanh(c), h write, state band fan-out."""
                s, r0, rows, og, t1l = ctx
                N = rows * 64
                cs = c_t[:, r0 * 64:r0 * 64 + N]
                if t1l is not None:
                    nc.vector.tensor_add(cs, cs, t1l[:])
                th = thp.tile([64, N], F32, tag="th")
                nc.scalar.activation(th[:], cs, ACT.Tanh)
                # h = tanh(c) * sigmoid(o) -> write buffer, padded layout
                nc.vector.tensor_mul(
                    hv_w[0:64, r0 + 1:r0 + 1 + rows, 1:65],
                    th[:].rearrange("p (r c) -> p r c", c=64),
                    og[0:64].rearrange("p (r c) -> p r c", c=64))
                # state fan-out: contiguous flat band copies
                b0 = (r0 + 1) * PW
                L = rows * PW
                hwf, h2f = h_w[:], h2_w[:]
                # hh upper: h shifted +2 cols (junk in pad cols, never read)
                nc.scalar.dma_start(hwf[64:128, b0:b0 + L - 2],
                                    hwf[0:64, b0 + 2:b0 + L])
                # hh2 lower: plain copy of h
                nc.scalar.dma_start(h2f[0:64, b0:b0 + L],
                                    hwf[0:64, b0:b0 + L])
                # hh2 upper: h shifted +2 rows (band lands 2 rows up)
                if r0 == 0:
                    # dst band [b0-2PW, ...) clipped at 0: drop PW elems
                    nc.scalar.dma_start(h2f[64:128, 0:L - PW],
                                        hwf[0:64, b0 + PW:b0 + L])
                else:
                    nc.scalar.dma_start(
                        h2f[64:128, b0 - 2 * PW:b0 + L - 2 * PW],
                        hwf[0:64, b0:b0 + L])

            x2_cur = load_x2(1)
            x2_nxt = load_x2(2)
            pend = None  # (ctx, h_w, h2_w, hv_w) of the previous tile
            for s in range(1, NSTEPS + 1):
                ph = "e" if s <= T else "d"
                rp = regions[s - 1]
                if s % 2 == 0:  # read buffers written at s-1
                    h_r, h2_r, h_w, h2_w = hhA, h2A, hhB, h2B
                else:
                    h_r, h2_r, h_w, h2_w = hhB, h2B, hhA, h2A
                hv_r, h2v_r, hv_w = gv(h_r), gv(h2_r), gv(h_w)

                if s > T + 1:
                    # prev decoder step's out conv reads all of step s-1's
                    # bands, including the pending last tile: flush first.
                    if pend is not None:
                        gate_phase2(*pend)
                        pend = None
                    emit_outconv(s - 1, hv_r, h2v_r)
                x2v = x2_cur[:].rearrange("p (r c) -> p r c", c=PW)

                r0 = 0
                while r0 < rp:
                    rows = min(8, rp - r0)
                    ctx = gate_phase1(s, ph, r0, rows, hv_r, h2v_r, x2v)
                    if pend is not None:
                        gate_phase2(*pend)
                    pend = (ctx, h_w, h2_w, hv_w)
                    r0 += 8

                if s + 2 <= NSTEPS:
                    x2_cur, x2_nxt = x2_nxt, load_x2(s + 2)
                else:
                    x2_cur = x2_nxt

            if pend is not None:
                gate_phase2(*pend)
            # out conv for the final decoder step (NSTEPS even -> B bufs)
            emit_outconv(NSTEPS, gv(hhB), gv(h2B))

    nc.compile()
    return nc


def _prep_core_inputs(core, enc_in, dec_in, enc_W, enc_b, dec_W, dec_b,
                      out_W, out_b, use_bf16=True):
    import ml_dtypes
    mm_np = ml_dtypes.bfloat16 if use_bf16 else np.float32
    b, half = core // 2, core % 2
    # gate permutation: [f, i, o, g]
    perm = np.concatenate([np.arange(0, 128), np.arange(192, 256),
                           np.arange(128, 192)])

    def prep_x2(x):
        x = x[b]  # [T, F, 64, 64]
        if half:
            x = x[:, :, ::-1, :]
        xp = np.zeros((T, F, PW, PW), np.float32)
        xp[:, :, 1:65, 1:65] = x
        flat = xp.reshape(T, F, PW * PW)
        x2 = np.empty((T, 72, X2LEN), np.float32)
        for tap in range(9):
            sh = (tap // 3) * PW + (tap % 3)
            x2[:, tap * 8:(tap + 1) * 8, :] = flat[:, :, sh:sh + X2LEN]
        return x2

    def prep_gateW(W, bias):
        Wf = W[:, :, ::-1, :] if half else W
        Wp = np.ascontiguousarray(Wf[perm]).astype(np.float64)
        bp = bias[perm].astype(np.float64)
        # double the g gate so sigmoid(2g) gives tanh via 2s-1
        Wp[192:256] *= 2.0
        bp[192:256] *= 2.0
        lx = np.zeros((128, 256))
        lx[0:72] = Wp[:, :F].transpose(2, 3, 1, 0).reshape(72, 256)
        lp = [np.concatenate([Wp[:, F:, k, 0].T, Wp[:, F:, k, 2].T], axis=0)
              for k in range(3)]  # [128, 256]
        lm = np.concatenate([Wp[:, F:, 0, 1].T, Wp[:, F:, 2, 1].T],
                            axis=0)  # [128, 256]
        ls = np.zeros((128, 256))
        ls[0:64] = Wp[:, F:, 1, 1].T
        return (lx, lp, lm, ls, bp[0:128].reshape(128, 1),
                bp[128:256].reshape(128, 1))

    ex, ep, em, es, eb0, eb1 = prep_gateW(enc_W, enc_b)
    dx, dp, dm, ds, db0, db1 = prep_gateW(dec_W, dec_b)
    oWf = out_W[:, :, ::-1, :] if half else out_W
    opad = np.zeros((128, 8))
    blk = {"xe": ex, "xd": dx, "me": em, "md": dm, "se": es, "sd": ds}
    for k in range(3):
        blk[f"pe{k}"] = ep[k]
        blk[f"pd{k}"] = dp[k]
    op = {}
    for k in range(3):
        op[f"op{k}"] = np.concatenate(
            [oWf[:, :, k, 0].T, oWf[:, :, k, 2].T], axis=0)  # [128, 8]
    op["om"] = np.concatenate(
        [oWf[:, :, 0, 1].T, oWf[:, :, 2, 1].T], axis=0)  # [128, 8]
    os_ = opad.copy()
    os_[0:64] = oWf[:, :, 1, 1].T
    op["os"] = os_

    wall = np.concatenate([blk[k] for k in _GBLK] +
                          [op[k] for k in _OBLK], axis=1)
    assert wall.shape == (128, _WCOLS)

    ball = np.zeros((128, 5), np.float32)
    ball[:, 0:1] = eb0
    ball[:, 1:2] = eb1
    ball[:, 2:3] = db0
    ball[:, 3:4] = db1
    for j in range(4):
        ball[32 * j:32 * j + 8, 4] = out_b

    x2_all = np.concatenate([prep_x2(enc_in), prep_x2(dec_in)], axis=0)

    return {"x2": np.ascontiguousarray(x2_all.astype(mm_np)),
            "wall": np.ascontiguousarray(wall.astype(mm_np)),
            "ball": np.ascontiguousarray(ball)}


def _install_trace_hook():
    """Shim antenv.axon_hooks for NTFF profiling (dev only)."""
    import contextlib
    import ctypes
    import types

    so = "/opt/axon/libaxon_pjrt.so"
    if "antenv.axon_hooks" in sys.modules or not os.path.exists(so):
        return
    lib = ctypes.CDLL(so)
    if not hasattr(lib, "axon_start_nrt_profile"):
        return
    lib.axon_start_nrt_profile.argtypes = [ctypes.POINTER(ctypes.c_int64),
                                           ctypes.c_size_t]
    lib.axon_start_nrt_profile.restype = ctypes.c_int64
    lib.axon_stop_nrt_profile.argtypes = [ctypes.c_char_p]
    lib.axon_stop_nrt_profile.restype = ctypes.c_int64

    def _mk():
        @contextlib.contextmanager
        def _hook(output_dir, device_ids):
            import jax
            jax.devices()
            if device_ids:
                ids = (ctypes.c_int64 * len(device_ids))(*device_ids)
                rc = lib.axon_start_nrt_profile(ids, len(device_ids))
            else:
                rc = lib.axon_start_nrt_profile(None, 0)
            if rc != 0:
                raise RuntimeError(f"axon_start_nrt_profile rc={rc}")
            try:
                yield
            finally:
                lib.axon_stop_nrt_profile(str(output_dir).encode())
        return _hook

    mod = types.ModuleType("antenv.axon_hooks")
    mod.get_axon_ntff_profile_hook = _mk
    sys.modules["antenv.axon_hooks"] = mod


def kernel(enc_in, dec_in, enc_W, enc_b, dec_W, dec_b, out_W, out_b):
    from concourse.bass_utils import run_bass_kernel_spmd

    trace = os.environ.get("KERNEL_TRACE", "") == "1"
    if trace:
        _install_trace_hook()

    use_bf16 = os.environ.get("KERNEL_DTYPE", "bf16") != "f32r"
    if "nc" not in _CACHE:
        _CACHE["nc"] = _build_program(use_bf16)
    nc = _CACHE["nc"]

    args = (np.asarray(enc_in, np.float32), np.asarray(dec_in, np.float32),
            np.asarray(enc_W, np.float32), np.asarray(enc_b, np.float32),
            np.asarray(dec_W, np.float32), np.asarray(dec_b, np.float32),
            np.asarray(out_W, np.float32), np.asarray(out_b, np.float32))
    in_maps = [_prep_core_inputs(c, *args, use_bf16=use_bf16)
               for c in range(NCORES)]

    res = run_bass_kernel_spmd(nc, in_maps, list(range(NCORES)), trace=trace)
    if trace:
        _CACHE["exec_time_ns"] = res.exec_time_ns

    B = enc_in.shape[0]
    out = np.empty((B, T, F, HS, WS), np.float32)
    for c in range(NCORES):
        b, half = c // 2, c % 2
        yc = res.results[c]["y"]  # [T, F, 32, 64]
        if half:
            out[b, :, :, 32:64, :] = yc[:, :, ::-1, :]
        else:
            out[b, :, :, 0:32, :] = yc
    return out
